# revision 46
# baseline (speedup 1.0000x reference)
"""Trainium2 Bass kernel for nn_CausalSelfAttention (B=1, S=2048, D=1024, H=16).

Tensor-parallel over heads across 8 NeuronCores: core c computes heads
(2c, 2c+1) end-to-end; the host sums the 8 bf16 partial outputs
(row-parallel Wout) and returns (y, v1) like the reference.

v2 layout (all matmul operands bf16, fp32 only in PSUM + norm chain):
  - qT/kT are [m, S] bf16 (m = 128 head dims = 2 heads x 64) computed as
    W_slice @ x.T with bf16 xT chunks as the moving operand.
  - v is projected DIRECTLY in transposed layout: per 128-key block,
    stationary xT chunk x moving WvT -> PSUM [kpos, vdim]; the value
    residual (lambda*v1, host-pretransposed) is added during PSUM
    evacuation into v_ext (with bf16 ones columns for the softmax
    denominator trick).
  - both RMS-norm scales are folded in early: q *= invq (with 1/sqrt(hd))
    and k *= invk right after RoPE, so the attention loop's Exp needs no
    per-partition scale AP.
  - scores are computed transposed [k, q]; causal diag mask is ADDED by
    the PE (identb @ maskb into the accumulating PSUM group). Exp output
    is bf16 and feeds the PV matmul directly.
  - output projection runs at the end in 2048-col chunks, partials
    written to HBM in bf16 and summed on the host.
"""

import os
import sys

import numpy as np

try:
    import concourse.bass as bass  # noqa: F401
except Exception:  # pragma: no cover
    for _p in ("/opt/trn_rl_repo", "/root/.axon_site/_ro/trn_rl_repo"):
        if os.path.isdir(_p) and _p not in sys.path:
            sys.path.insert(0, _p)

import concourse.bacc as bacc
import concourse.bass as bass
import concourse.mybir as mybir
import concourse.tile as tile
from concourse import bass_utils

S = 2048
D = 1024
NH = 16
HD = 64
NCORES = 8
M = (NH // NCORES) * HD  # 128 m-dims per core (2 heads)
NCHUNK = D // 128        # 8 contraction chunks for the projections
NKC = S // 128           # 16 key chunks
HALF = S // 2            # attention processed in q-halves of 1024

F32 = mybir.dt.float32
F32R = mybir.dt.float32r
BF16 = mybir.dt.bfloat16
AF = mybir.ActivationFunctionType

EPS = float(np.finfo(np.float32).eps)
NEG = -1e30


def _splits(n0, n1, step=512):
    """[n0, n1) split at absolute multiples of `step`."""
    out = []
    a = n0
    while a < n1:
        b = min(n1, (a // step + 1) * step)
        out.append((a, b))
        a = b
    return out


def r(ap):
    return ap.bitcast(F32R)


def _emit(tc, io):
    nc = tc.nc
    ctx_pools = []

    def pool(*a, **k):
        return tc.alloc_tile_pool(*a, **k)

    consts = pool(name="consts", bufs=1)
    wpool = pool(name="wpool", bufs=1)
    persist = pool(name="persist", bufs=1)
    work = pool(name="work", bufs=1)
    ctx_pools += [consts, wpool, persist, work]

    # ---- DMA: first what the q projection needs, then the rest ------
    # (all host arrays pre-transposed so per-partition runs are contiguous)
    wq_sb = wpool.tile([128, NCHUNK, 128], BF16, name="wq")
    nc.sync.dma_start(out=wq_sb, in_=io["wqT"].ap())
    xtp = pool(name="xt", bufs=1)
    xt_sb = xtp.tile([128, NCHUNK, S], BF16)
    xt_dram = io["xT"].ap()
    nc.sync.dma_start(out=xt_sb[:, 0, 0:HALF], in_=xt_dram[:, 0, 0:HALF])
    nc.sync.dma_start(out=xt_sb[:, 0, HALF:S], in_=xt_dram[:, 0, HALF:S])
    for c in range(1, NCHUNK):
        nc.sync.dma_start(out=xt_sb[:, c, :], in_=xt_dram[:, c, :])
    wk_sb = wpool.tile([128, NCHUNK, 128], BF16, name="wk")
    nc.sync.dma_start(out=wk_sb, in_=io["wkT"].ap())
    wv_sb = wpool.tile([128, NCHUNK, 128], BF16, name="wv")
    nc.sync.dma_start(out=wv_sb, in_=io["wvT"].ap())

    cosT = consts.tile([128, S], BF16)
    nc.sync.dma_start(out=cosT, in_=io["cosT"].ap())
    sinTs = consts.tile([128, S], BF16)
    nc.sync.dma_start(out=sinTs, in_=io["sinTs"].ap())
    v1_sb = wpool.tile([128, NKC, 128], BF16, name="v1T")
    nc.sync.dma_start(out=v1_sb, in_=io["v1T"].ap())
    identb = consts.tile([128, 128], BF16)
    nc.sync.dma_start(out=identb, in_=io["identb"].ap())
    maskb = consts.tile([128, 128], BF16)
    nc.sync.dma_start(out=maskb, in_=io["maskb"].ap())
    c4 = consts.tile([4, 2], F32)
    nc.sync.dma_start(out=c4, in_=io["c4"].ap())
    ind8 = consts.tile([128, 8], BF16)
    nc.sync.dma_start(out=ind8, in_=io["ind8"].ap())
    ind4Q = consts.tile([4, 128], BF16)
    nc.sync.dma_start(out=ind4Q, in_=io["ind4Q"].ap())
    identf = consts.tile([4, 4], F32)
    nc.sync.dma_start(out=identf, in_=io["identf"].ap())
    ind2B = consts.tile([2, 64], BF16)
    nc.sync.dma_start(out=ind2B, in_=io["ind2B"].ap())
    wo_sb = wpool.tile([128, D], BF16, name="wo")

    # ---- persistent activations -------------------------------------
    q_fin = persist.tile([128, S], BF16)
    k_fin = persist.tile([128, S], BF16)
    v_ext = persist.tile([128, NKC, 132], BF16)  # [k, chunk, 66*h] w/ ones cols
    y2T = persist.tile([128, S], BF16)

    sq_q = work.tile([128, S], BF16, tag="sq", name="sq_q", bufs=2)
    sq_k = work.tile([128, S], BF16, tag="sq", name="sq_k", bufs=2)
    swp_q = work.tile([128, S], BF16, tag="swp", name="swp_q", bufs=2)
    swp_k = work.tile([128, S], BF16, tag="swp", name="swp_k", bufs=2)
    sqrt_sb = work.tile([4, S], F32, name="sqrt_sb")
    invT_sb = work.tile([128, 4 * NKC], F32, name="invT_sb")
    inv4 = work.tile([4, S], F32, name="inv4")
    inv4b = work.tile([4, S], BF16, name="inv4b")

    ones32 = io["ones16"].ap().rearrange("p (a b) -> p a b", b=2)
    nc.sync.dma_start(out=v_ext[:, :, 64:66], in_=ones32)
    nc.sync.dma_start(out=v_ext[:, :, 130:132], in_=ones32)

    # ================= phase A: q/k projections ======================
    # chunk-interleaved: each xt chunk feeds both projections as soon as
    # its DMA lands, so the PE tracks the DMA stream instead of waiting
    pqkv = pool(name="pqkv", bufs=2, space="PSUM")
    ps_q = pqkv.tile([128, S], F32, tag="qkv", name="ps_q")
    ps_k = pqkv.tile([128, S], F32, tag="qkv", name="ps_k")
    for c in range(NCHUNK):
        for ps, w in ((ps_q, wq_sb), (ps_k, wk_sb)):
            for s0, s1 in _splits(0, S):
                nc.tensor.matmul(
                    ps[:, s0:s1],
                    w[:, c, :],
                    xt_sb[:, c, s0:s1],
                    start=(c == 0),
                    stop=(c == NCHUNK - 1),
                )

    nc.scalar.activation(out=sq_q, in_=ps_q, func=AF.Square)  # ACT: squares
    nc.scalar.activation(out=sq_k, in_=ps_k, func=AF.Square)
    nc.vector.tensor_copy(out=q_fin, in_=ps_q)                # DVE: evac bf16
    nc.vector.tensor_copy(out=k_fin, in_=ps_k)
    for d0, s0 in ((0, 32), (32, 0), (64, 96), (96, 64)):
        nc.sync.dma_start(out=swp_q[d0:d0 + 32, :], in_=q_fin[s0:s0 + 32, :])
    for d0, s0 in ((0, 32), (32, 0), (64, 96), (96, 64)):
        nc.sync.dma_start(out=swp_k[d0:d0 + 32, :], in_=k_fin[s0:s0 + 32, :])
    nc.vector.tensor_mul(out=q_fin, in0=q_fin, in1=cosT)
    nc.vector.tensor_mul(out=swp_q, in0=swp_q, in1=sinTs)
    nc.vector.tensor_add(out=q_fin, in0=q_fin, in1=swp_q)
    nc.vector.tensor_mul(out=k_fin, in0=k_fin, in1=cosT)
    nc.vector.tensor_mul(out=swp_k, in0=swp_k, in1=sinTs)
    nc.vector.tensor_add(out=k_fin, in0=k_fin, in1=swp_k)

    pqkv.release()

    # ============ phase B: v transposed-proj + norms + scales ========
    pvt = pool(name="pvt", bufs=4, space="PSUM")
    pnb = pool(name="pnb", bufs=1, space="PSUM")

    def vblock(t, vpool, tag="vt"):
        ps_v = vpool.tile([128, 128], F32, tag=tag, name=f"vt{t}")
        for c in range(NCHUNK):
            nc.tensor.matmul(
                ps_v,
                xt_sb[:, c, 128 * t:128 * t + 128],
                wv_sb[:, c, :],
                start=(c == 0),
                stop=(c == NCHUNK - 1),
            )
        # evacuate with the value-residual add (GpSimd can't read PSUM);
        # one strided DVE op covers both 64-col head groups
        vdst = v_ext[:, t, 0:132].rearrange("p (a b) -> p a b", b=66)[:, :, 0:64]
        vsrc = ps_v.rearrange("p (a b) -> p a b", b=64)
        v1s = v1_sb[:, t, :].rearrange("p (a b) -> p a b", b=64)
        nc.vector.tensor_add(out=vdst, in0=vsrc, in1=v1s)

    # RMS-norm sums on the PE (indicator matmul over partitions)
    ps_norm = pnb.tile([4, S], F32, tag="nb")
    for s0, s1 in _splits(0, S):
        nc.tensor.matmul(ps_norm[:, s0:s1], ind8[:, 0:4], sq_q[:, s0:s1],
                         start=True, stop=False)
    for s0, s1 in _splits(0, S):
        nc.tensor.matmul(ps_norm[:, s0:s1], ind8[:, 4:8], sq_k[:, s0:s1],
                         start=False, stop=True)
    nc.scalar.activation(out=sqrt_sb, in_=ps_norm, func=AF.Sqrt,
                         bias=c4[:, 1:2], scale=c4[:, 0:1])
    nc.vector.reciprocal_approx_fast(out=inv4, in_=sqrt_sb)
    nc.scalar.copy(out=inv4b, in_=inv4)

    for t in range(4):
        vblock(t, pvt)
    pnb.release()

    # q-side scale: broadcast to 64-partition groups and multiply.
    # k-side scale is NOT folded into k_fin: it rides the Exp scale AP
    # (per-partition = key position) via invT below.
    pbc = pool(name="pbc", bufs=2, space="PSUM")

    # invT[kpos, 4t+r] = inv4[r, 128t+kpos]  (PE transposes)
    ps_invT = pbc.tile([128, 4 * NKC], F32, tag="it", name="ps_invT")
    for t in range(NKC):
        nc.tensor.transpose(ps_invT[:, 4 * t:4 * t + 4],
                            inv4[:, 128 * t:128 * t + 128], identf[0:4, 0:4])
    nc.scalar.copy(out=invT_sb, in_=ps_invT)

    def scale_q_half(ph):
        p0 = HALF * ph
        T = pbc.tile([128, HALF], F32, tag="bc", bufs=1, name=f"bc{ph}")
        for l0, l1 in _splits(0, HALF):
            nc.tensor.matmul(T[:, l0:l1], ind4Q, inv4b[:, p0 + l0:p0 + l1],
                             start=True, stop=True)
        nc.vector.tensor_mul(out=q_fin[:, p0:p0 + HALF],
                             in0=q_fin[:, p0:p0 + HALF], in1=T)

    scale_q_half(0)
    scale_q_half(1)
    for t in range(4, 8):
        vblock(t, pvt)
    pbc.release()
    pvt.release()
    # v blocks 8-15 are emitted as PE filler inside the Hf0 attention
    # loop (their v_ext slices are only needed by Hf1), so xt_sb must
    # stay alive through attention emission.

    # ================= phase C: causal attention ======================
    late = pool(name="late", bufs=1)
    pattn = pool(name="pattn", bufs=2, space="PSUM")
    outp = io["outp"].ap()

    yts = {}
    pend = {}  # Hf -> [(ex, q0, qn, j) x 2 heads] awaiting their PV matmuls

    def flush_pv(Hf):
        """Emit the PV matmuls for the pending (previous) j block."""
        q0h = HALF * Hf
        njc = 8 * (Hf + 1)
        for h, (ex, q0, qn, j) in enumerate(pend.pop(Hf, [])):
            ystart = q0 - q0h
            nxt = max(0, 128 * (j + 1) - q0h)  # next strip's local start
            for w0, w1 in _splits(ystart, HALF):
                nc.tensor.matmul(
                    yts[Hf][h][:, w0:w1],
                    v_ext[:, j, 66 * h:66 * h + 66],
                    ex[:, w0 - ystart:w1 - ystart],
                    start=(j == 0),
                    stop=(j == njc - 1) or (w1 <= nxt),
                )

    def attn_jrange(Hf, j_lo, j_hi):
        """Software-pipelined: PV(j-1) is emitted after scores(j) so the PE
        never head-of-line blocks on Exp(j)."""
        q0h = HALF * Hf
        for j in range(j_lo, j_hi):
            pss = []
            q0 = max(q0h, 128 * j)
            qn = q0h + HALF - q0
            diag = (q0 == 128 * j)
            for h in range(2):
                ps_s = pattn.tile([128, HALF], F32, tag="sc", name=f"sc{Hf}_{j}_{h}")
                pss.append(ps_s)
                for idx, (l0, l1) in enumerate(_splits(0, qn)):
                    nc.tensor.matmul(
                        ps_s[:, l0:l1],
                        k_fin[64 * h:64 * h + 64, 128 * j:128 * j + 128],
                        q_fin[64 * h:64 * h + 64, q0 + l0:q0 + l1],
                        start=True, stop=not (diag and idx == 0),
                    )
                if diag:  # leading block is on the diagonal: add causal bias
                    nc.tensor.matmul(ps_s[:, 0:128], identb, maskb,
                                     start=False, stop=True)
            flush_pv(Hf)
            if Hf == 0:  # PE filler: second-half v blocks (needed by Hf1)
                vblock(8 + j, pattn, tag="sc")
            elif 4 <= j < 12:  # PE filler: output projection of q-half 0
                emit_oc(0, j - 4, "sc")
            elif j == 13:
                # strip [1024:1536) of y finalized at j=11: start its
                # denominator chain while attention continues
                den_strip(1, 0)
            elif j == 14:
                den_strip_post(1, 0)
                for oc in range(4):
                    emit_oc_strip(1, oc, 0, "sc")
            elif j == 15:
                for oc in range(4, 8):
                    emit_oc_strip(1, oc, 0, "sc")
            exs = []
            for h in range(2):
                ex = late.tile([128, HALF], BF16, tag="ex", bufs=4,
                               name=f"ex{Hf}_{j}_{h}")
                nc.scalar.activation(out=ex[:, :qn], in_=pss[h][:, :qn],
                                     func=AF.Exp, bias=0.0,
                                     scale=invT_sb[:, 4 * j + 2 + h:4 * j + 3 + h])
                exs.append((ex, q0, qn, j))
            pend[Hf] = exs

    den_state = {}

    def den_evac(Hf):
        """yts -> SBUF f32 (includes den rows), freeing the yt ring."""
        st = []
        for h in range(2):
            ys = late.tile([66, HALF], F32, tag="ys", bufs=2, name=f"ys{Hf}_{h}")
            if h == 0:
                nc.scalar.copy(out=ys, in_=yts[Hf][h])
            else:  # balance the two engines
                nc.vector.tensor_copy(out=ys, in_=yts[Hf][h])
            st.append(ys)
        den_state[Hf] = st

    def den_recip(Hf):
        """reciprocal of the denominator rows + bf16 copy for the PE.
        (reciprocal_approx_fast misreads partition-offset inputs, so the
        den rows are first copied into a partition-0-based tile)"""
        st = []
        for h in range(2):
            ys = den_state[Hf][h]
            den = late.tile([2, HALF], F32, tag="den", bufs=2, name=f"den{Hf}_{h}")
            if h == 0:
                nc.scalar.copy(out=den, in_=ys[64:66, :])
            else:
                nc.vector.tensor_copy(out=den, in_=ys[64:66, :])
            scr = late.tile([2, HALF], F32, tag="scr", bufs=2, name=f"scr{Hf}_{h}")
            nc.vector.reciprocal_approx_fast(out=scr, in_=den)
            scrb = late.tile([2, HALF], BF16, tag="scrb", bufs=2,
                             name=f"scrb{Hf}_{h}")
            if h == 0:
                nc.scalar.copy(out=scrb, in_=scr)
            else:
                nc.vector.tensor_copy(out=scrb, in_=scr)
            if "dbg_ys" in io and Hf == 0 and h == 0:
                nc.sync.dma_start(out=io["dbg_ys"].ap(), in_=ys)
                nc.sync.dma_start(out=io["dbg_scr"].ap(), in_=scr)
            st.append((ys, scrb))
        den_state[Hf] = st

    def den_post(Hf):
        """invbc bcast (PE) + final y2T scale (DVE)."""
        q0h = HALF * Hf
        invbc = pattn.tile([128, HALF], F32, tag="sc", name=f"ivb{Hf}")
        for h in range(2):
            ys, scrb = den_state[Hf][h]
            for l0, l1 in _splits(0, HALF):
                nc.tensor.matmul(invbc[64 * h:64 * h + 64, l0:l1], ind2B,
                                 scrb[:, l0:l1], start=True, stop=True)
            nc.vector.tensor_mul(
                out=y2T[64 * h:64 * h + 64, q0h:q0h + HALF],
                in0=ys[0:64, :], in1=invbc[64 * h:64 * h + 64, :])

    def emit_oc(half, oc, tag, act_evac=False):
        h0 = HALF * half
        ps_o = pattn.tile([128, HALF], F32, tag=tag, name=f"o{oc}_{half}")
        for l0, l1 in _splits(0, HALF):
            nc.tensor.matmul(ps_o[:, l0:l1],
                             wo_sb[:, 128 * oc:128 * oc + 128],
                             y2T[:, h0 + l0:h0 + l1], start=True, stop=True)
        osb = late.tile([128, HALF], BF16, tag="osb", bufs=4,
                        name=f"osb{oc}_{half}")
        if act_evac:
            nc.scalar.copy(out=osb, in_=ps_o)
        else:
            nc.vector.tensor_copy(out=osb, in_=ps_o)
        nc.sync.dma_start(out=outp[oc][:, h0:h0 + HALF], in_=osb)

    strip_state = {}

    def den_strip(Hf, sp):
        """denominator chain for a finalized 512-col strip of yts[Hf]."""
        c0 = 512 * sp
        st = []
        for h in range(2):
            ys = late.tile([66, 512], F32, tag="yss", bufs=2,
                           name=f"yss{Hf}_{sp}_{h}")
            den = late.tile([2, 512], F32, tag="dens", bufs=2,
                            name=f"dens{Hf}_{sp}_{h}")
            if h == 0:
                nc.scalar.copy(out=ys, in_=yts[Hf][h][:, c0:c0 + 512])
                nc.scalar.copy(out=den, in_=yts[Hf][h][64:66, c0:c0 + 512])
            else:
                nc.vector.tensor_copy(out=ys, in_=yts[Hf][h][:, c0:c0 + 512])
                nc.vector.tensor_copy(out=den, in_=yts[Hf][h][64:66, c0:c0 + 512])
            scr = late.tile([2, 512], F32, tag="scrs", bufs=2,
                            name=f"scrs{Hf}_{sp}_{h}")
            nc.vector.reciprocal_approx_fast(out=scr, in_=den)
            scrb = late.tile([2, 512], BF16, tag="scrbs", bufs=2,
                             name=f"scrbs{Hf}_{sp}_{h}")
            if h == 0:
                nc.scalar.copy(out=scrb, in_=scr)
            else:
                nc.vector.tensor_copy(out=scrb, in_=scr)
            st.append((ys, scrb))
        strip_state[(Hf, sp)] = st

    def den_strip_post(Hf, sp):
        q0h = HALF * Hf
        c0 = 512 * sp
        ivb = pattn.tile([128, 512], F32, tag="sc", name=f"ivbs{Hf}_{sp}")
        for h in range(2):
            ys, scrb = strip_state[(Hf, sp)][h]
            nc.tensor.matmul(ivb[64 * h:64 * h + 64, :], ind2B, scrb,
                             start=True, stop=True)
            nc.vector.tensor_mul(
                out=y2T[64 * h:64 * h + 64, q0h + c0:q0h + c0 + 512],
                in0=ys[0:64, :], in1=ivb[64 * h:64 * h + 64, :])

    def emit_oc_strip(half, oc, sp, tag):
        g0 = HALF * half + 512 * sp
        ps_o = pattn.tile([128, 512], F32, tag=tag, name=f"os{oc}_{half}_{sp}")
        nc.tensor.matmul(ps_o, wo_sb[:, 128 * oc:128 * oc + 128],
                         y2T[:, g0:g0 + 512], start=True, stop=True)
        osb = late.tile([128, 512], BF16, tag="osbs", bufs=4,
                        name=f"osbs{oc}_{half}_{sp}")
        if oc % 2 == 0:
            nc.vector.tensor_copy(out=osb, in_=ps_o)
        else:
            nc.scalar.copy(out=osb, in_=ps_o)
        nc.sync.dma_start(out=outp[oc][:, g0:g0 + 512], in_=osb)

    yts[0] = [pattn.tile([66, HALF], F32, tag="yt", name=f"yt0_{h}")
              for h in range(2)]
    attn_jrange(0, 0, 8)
    nc.sync.dma_start(out=wo_sb, in_=io["woT"].ap())
    yts[1] = [pattn.tile([66, HALF], F32, tag="yt", name=f"yt1_{h}")
              for h in range(2)]
    attn_jrange(1, 0, 2)
    flush_pv(0)
    den_evac(0)
    den_recip(0)
    attn_jrange(1, 2, 4)
    den_post(0)
    attn_jrange(1, 4, 16)
    flush_pv(1)
    den_strip(1, 1)
    den_strip_post(1, 1)
    for oc in range(8):
        emit_oc_strip(1, oc, 1, "sc" if oc % 2 == 0 else "yt")

    pattn.release()

    if "dbg_q" in io:
        nc.sync.dma_start(out=io["dbg_q"].ap(), in_=q_fin)
        nc.sync.dma_start(out=io["dbg_k"].ap(), in_=k_fin)
        nc.sync.dma_start(out=io["dbg_vext"].ap(), in_=v_ext)
        nc.sync.dma_start(out=io["dbg_inv4"].ap(), in_=inv4)
        nc.sync.dma_start(out=io["dbg_y2T"].ap(), in_=y2T)

    late.release()
    xtp.release()
    for p in reversed(ctx_pools):
        p.release()


_CACHE = {}


def _build(debug_taps=False):
    key = ("nc", debug_taps)
    if key in _CACHE:
        return _CACHE[key]
    nc = bacc.Bacc("TRN2", target_bir_lowering=False, debug=False,
                   enable_asserts=True, num_devices=NCORES)
    io = {}
    io["xT"] = nc.dram_tensor("xT", [128, NCHUNK, S], BF16, kind="ExternalInput")
    io["cosT"] = nc.dram_tensor("cosT", [128, S], BF16, kind="ExternalInput")
    io["sinTs"] = nc.dram_tensor("sinTs", [128, S], BF16, kind="ExternalInput")
    io["identb"] = nc.dram_tensor("identb", [128, 128], BF16, kind="ExternalInput")
    io["maskb"] = nc.dram_tensor("maskb", [128, 128], BF16, kind="ExternalInput")
    io["c4"] = nc.dram_tensor("c4", [4, 2], F32, kind="ExternalInput")
    io["ind8"] = nc.dram_tensor("ind8", [128, 8], BF16, kind="ExternalInput")
    io["ind4Q"] = nc.dram_tensor("ind4Q", [4, 128], BF16, kind="ExternalInput")
    io["identf"] = nc.dram_tensor("identf", [4, 4], F32, kind="ExternalInput")
    io["ind2B"] = nc.dram_tensor("ind2B", [2, 64], BF16, kind="ExternalInput")
    io["ones16"] = nc.dram_tensor("ones16", [128, 32], BF16, kind="ExternalInput")
    io["wqT"] = nc.dram_tensor("wqT", [128, NCHUNK, 128], BF16, kind="ExternalInput")
    io["wkT"] = nc.dram_tensor("wkT", [128, NCHUNK, 128], BF16, kind="ExternalInput")
    io["wvT"] = nc.dram_tensor("wvT", [128, NCHUNK, 128], BF16, kind="ExternalInput")
    io["woT"] = nc.dram_tensor("woT", [M, D], BF16, kind="ExternalInput")
    io["v1T"] = nc.dram_tensor("v1T", [128, NKC, 128], BF16, kind="ExternalInput")
    io["outp"] = nc.dram_tensor("outp", [8, 128, S], BF16, kind="ExternalOutput")
    if debug_taps:
        io["dbg_q"] = nc.dram_tensor("dbg_q", [128, S], BF16, kind="ExternalOutput")
        io["dbg_k"] = nc.dram_tensor("dbg_k", [128, S], BF16, kind="ExternalOutput")
        io["dbg_vext"] = nc.dram_tensor("dbg_vext", [128, NKC, 132], BF16, kind="ExternalOutput")
        io["dbg_inv4"] = nc.dram_tensor("dbg_inv4", [4, S], F32, kind="ExternalOutput")
        io["dbg_y2T"] = nc.dram_tensor("dbg_y2T", [128, S], BF16, kind="ExternalOutput")
        io["dbg_ys"] = nc.dram_tensor("dbg_ys", [66, HALF], F32, kind="ExternalOutput")
        io["dbg_scr"] = nc.dram_tensor("dbg_scr", [2, HALF], F32, kind="ExternalOutput")

    with tile.TileContext(nc) as tc:
        _emit(tc, io)
    nc.compile()
    _CACHE[key] = nc
    return nc


def _host_prep(x, v1, Wq, Wk, Wv, Wout, lambdas):
    """Build per-core input maps (bf16 operands)."""
    import ml_dtypes
    bf = ml_dtypes.bfloat16

    x = np.asarray(x, np.float32).reshape(S, D)
    v1 = np.asarray(v1, np.float32).reshape(S, D)
    Wq = np.asarray(Wq, np.float32)
    Wk = np.asarray(Wk, np.float32)
    Wv = np.asarray(Wv, np.float32)
    Wout = np.asarray(Wout, np.float32)
    lam = np.float32(np.asarray(lambdas))

    # [pi, chunk, s]: partition-contiguous chunks of x.T
    xT = np.ascontiguousarray(
        x.T.reshape(NCHUNK, 128, S).transpose(1, 0, 2)).astype(bf)

    def wprep(w):  # [D, M] -> [pi, chunk, m]
        return np.ascontiguousarray(
            w.reshape(NCHUNK, 128, M).transpose(1, 0, 2)).astype(bf)

    inv_freq = (np.float32(1.0)
                / np.power(np.float32(10000.0),
                           np.arange(0, HD, 2, dtype=np.float32) / np.float32(HD)))
    t = np.arange(S, dtype=np.float32)
    freqs = np.outer(t, inv_freq).astype(np.float32)       # [S, 32]
    cos = np.cos(freqs).T                                   # [32, S]
    sin = np.sin(freqs).T
    cosT = np.ascontiguousarray(np.tile(cos, (4, 1))).astype(bf)
    sinTs = np.ascontiguousarray(
        np.concatenate([sin, -sin, sin, -sin], axis=0)).astype(bf)

    identb = np.eye(128, dtype=bf)
    kk, qq = np.meshgrid(np.arange(128), np.arange(128), indexing="ij")
    maskb = np.where(qq >= kk, 0.0, NEG).astype(bf)

    c4 = np.array([[1.0, 64.0 * EPS],
                   [1.0, 64.0 * EPS],
                   [1.0 / 64.0, EPS],
                   [1.0 / 64.0, EPS]], dtype=np.float32)
    # norms rows: 0-1 q (h0, h1), 2-3 k (h0, h1); q matmul uses cols 0:4,
    # k matmul uses cols 4:8 (slice-local column index = psum row)
    ind8 = np.zeros((128, 8), dtype=np.float32)
    ind8[0:64, 0] = 1.0     # q h0 -> norms row 0
    ind8[64:128, 1] = 1.0   # q h1 -> norms row 1
    ind8[0:64, 6] = 1.0     # k h0 -> norms row 2
    ind8[64:128, 7] = 1.0   # k h1 -> norms row 3
    ind8 = ind8.astype(bf)

    ind4Q = np.zeros((4, 128), dtype=np.float32)
    ind4Q[0, 0:64] = 1.0    # inv4 row 0 = q h0 scale
    ind4Q[1, 64:128] = 1.0  # inv4 row 1 = q h1 scale
    identf = np.eye(4, dtype=np.float32)
    ind2B = np.zeros((2, 64), dtype=np.float32)
    ind2B[0, :] = 1.0       # den row 0 -> all 64 partitions of the head
    ind4Q = ind4Q.astype(bf)
    ind2B = ind2B.astype(bf)

    ones16 = np.ones((128, 32), dtype=bf)

    shared = dict(xT=xT, cosT=cosT, sinTs=sinTs, identb=identb,
                  maskb=maskb, c4=c4, ind8=ind8, ind4Q=ind4Q, identf=identf,
                  ind2B=ind2B, ones16=ones16)

    in_maps = []
    for c in range(NCORES):
        sl = slice(128 * c, 128 * c + 128)
        m = dict(shared)
        m["wqT"] = wprep(Wq[sl, :].T)
        m["wkT"] = wprep(Wk[sl, :].T)
        m["wvT"] = wprep(((np.float32(1.0) - lam) * Wv[sl, :]).T)
        m["woT"] = np.ascontiguousarray(Wout[:, sl].T).astype(bf)
        m["v1T"] = np.ascontiguousarray(
            (lam * v1[:, sl]).reshape(NKC, 128, M).transpose(1, 0, 2)).astype(bf)
        in_maps.append(m)
    return in_maps


def run(inputs, trace=False, debug_taps=False):
    nh = int(np.asarray(inputs["n_heads"]))
    assert nh == NH, f"kernel compiled for n_heads={NH}, got {nh}"
    nc = _build(debug_taps)
    in_maps = _host_prep(inputs["x"], inputs["v1"], inputs["Wq"], inputs["Wk"],
                         inputs["Wv"], inputs["Wout"], inputs["lambdas"])
    res = bass_utils.run_bass_kernel_spmd(
        nc, in_maps, core_ids=list(range(NCORES)), trace=trace)
    outT = np.zeros((D, S), dtype=np.float32)
    for c in range(NCORES):
        outT += res.results[c]["outp"].astype(np.float32).reshape(D, S)
    y = np.ascontiguousarray(outT.T).reshape(1, S, D).astype(np.float32)
    v1 = np.asarray(inputs["v1"], np.float32).reshape(1, S, D)
    return (y, v1), res


def kernel(**inputs):
    outs, _ = run(inputs, trace=False)
    return outs
